# revision 46
# baseline (speedup 1.0000x reference)
"""Trainium2 Bass kernel for 8-head MultiHeadAttention (B=2, S=4096, E=512).

Sharding: 8 cores = 2 batches x 4 query-row chunks of 1024. Each core computes
all 8 heads for its (batch, q-range).

Algebraic folds (host-side, exact):
  - G = Wq^T Wk folded into the q projection: scores = (Xq G) Xk^T, so the
    K projection disappears and raw Xk columns are the scores stationary.
  - Wv folded into Wo: the attention-value matmul uses raw `value` columns
    (plus a ones column for the softmax denominator), and the output
    projection uses Wo''_h = Wv^T Wo_h^T.

Layout: scores are built TRANSPOSED ([k partitions, q free]); the
attention-value matmul is emitted per 128-q tile with the P^T chunk as the
stationary operand, producing ctx NON-transposed ([q partitions, 65 free])
so the softmax denominator is a per-partition scalar (cheap tensor_scalar
normalize), then a PE transpose restores ctx^T for the output projection.
Softmax runs without max-subtraction (scores/8 are bounded); the mask is
applied multiplicatively after exp (bf16 on DVE).
"""
import sys
for _p in ('/root/.axon_site/_ro/trn_rl_repo', '/opt/trn_rl_repo'):
    if _p not in sys.path:
        sys.path.append(_p)

import numpy as np
import ml_dtypes

import concourse.bass as bass
import concourse.tile as tile
from concourse import bacc, mybir
from concourse import bass_utils
import concourse.dve_ops as dve_ops_mod
from concourse.dve_ops import DveOp
from concourse.dve_spec import Spec, Src0, Src1, C0, C1, C2, One, sq
from concourse.dve_spec import lower as dve_lower
from concourse.dve_uop import DveOpSpec

F32 = mybir.dt.float32
BF16 = mybir.dt.bfloat16
AF = mybir.ActivationFunctionType
ALU = mybir.AluOpType

N_CORES = 8
B, S, E, H, DH = 2, 4096, 512, 8, 64
QLEN = S // 4          # 1024 q rows per core
KC = S // 128          # 32 k chunks
QW = QLEN // 512       # 2 q windows of 512

_CACHE = {}

# exp(s/8) on the DVE in two table-programmed ops: p = poly3(s) ~ e^(s/1024)
# (|s| < ~50 so |s|/1024 < 0.05 and a cubic is exact to ~1e-7), then seven
# squarings p^128 = e^(s/8), with the {0,1} mask multiplied in the same pass.
_EB1, _EB2, _EB3 = 1.0 / 1024, 1.0 / (2 * 1024 ** 2), 1.0 / (6 * 1024 ** 3)


def _register_exp_ops():
    if "EXP1024_POLY_ANT" in dve_ops_mod._SUB_OPCODE_FOR_NAME:
        by = {op.name: op for op in dve_ops_mod.OPS}
        return by["EXP1024_POLY_ANT"], by["POW128_MASK_ANT"]

    def ref1(in0, in1, s0, s1, imm2):
        x = in0.astype(np.float32)
        return (((x * np.float32(imm2) + np.float32(s1)) * x + np.float32(s0))
                * x + np.float32(1.0)).astype(np.float32)

    spec1 = Spec(body=((Src0 * C2 + C1) * Src0 + C0) * Src0 + One,
                 reference=ref1)

    b = Src0
    for _ in range(7):
        b = sq(b)

    def ref2(in0, in1, s0, s1, imm2):
        x = in0.astype(np.float32)
        for _ in range(7):
            x = (x * x).astype(np.float32)
        return (x * in1).astype(np.float32)

    spec2 = Spec(body=b * Src1, reference=ref2)

    ops = []
    for name, spec, rd1 in [("EXP1024_POLY_ANT", spec1, False),
                            ("POW128_MASK_ANT", spec2, True)]:
        row = max(dve_ops_mod._SUB_OPCODE_FOR_NAME.values()) + 1
        shas = {}
        for ver in ("v3", "v4"):
            uops = dve_lower(spec, ver=ver)
            shas[ver] = DveOpSpec(name=name, opcode=row, uops=uops,
                                  rd1_en=rd1).sha(ver)
        op = DveOp(name, spec, subdim=False, uops_sha=shas)
        dve_ops_mod.OPS.append(op)
        dve_ops_mod._SUB_OPCODE_FOR_NAME[name] = row
        dve_ops_mod.CUSTOM_DVE_SPECS[name] = spec
        ops.append(op)
    return ops[0], ops[1]


def _build_module():
    nc = bacc.Bacc("TRN2", target_bir_lowering=False, debug=False,
                   enable_asserts=True, num_devices=N_CORES)

    xqT = nc.dram_tensor("xqT", [E, QLEN], BF16, kind="ExternalInput").ap()
    xkT = nc.dram_tensor("xkT", [E, S], BF16, kind="ExternalInput").ap()
    valp = nc.dram_tensor("valp", [S, H * 65], BF16, kind="ExternalInput").ap()
    maskT = nc.dram_tensor("maskT", [S, QLEN], BF16, kind="ExternalInput").ap()
    bdG = nc.dram_tensor("bdG", [128, 128], BF16, kind="ExternalInput").ap()
    ident = nc.dram_tensor("ident", [128, 128], BF16, kind="ExternalInput").ap()
    woT = nc.dram_tensor("woT", [E, E], BF16, kind="ExternalInput").ap()
    bo_b = nc.dram_tensor("bo_b", [128, E], F32, kind="ExternalInput").ap()
    out = nc.dram_tensor("out", [QLEN, E], F32, kind="ExternalOutput").ap()

    with tile.TileContext(nc) as tc:
        _emit(tc, nc, xqT, xkT, valp, maskT, bdG, ident, woT, bo_b, out)

    nc.compile()
    return nc


def _emit(tc, nc, xqT, xkT, valp, maskT, bdG, ident, woT, bo_b, out):
    from contextlib import ExitStack
    ctx = ExitStack()
    op_poly, op_pow = _register_exp_ops()
    const = ctx.enter_context(tc.tile_pool(name="const", bufs=1))
    xkst = ctx.enter_context(tc.tile_pool(name="xkst", bufs=4))
    qgp = ctx.enter_context(tc.tile_pool(name="qgp", bufs=2))
    ptp = ctx.enter_context(tc.tile_pool(name="ptp", bufs=12))
    pep = ctx.enter_context(tc.tile_pool(name="pep", bufs=3))
    unp = ctx.enter_context(tc.tile_pool(name="unp", bufs=10))
    rcp = ctx.enter_context(tc.tile_pool(name="rcp", bufs=8))
    ospool = ctx.enter_context(tc.tile_pool(name="osb", bufs=2))
    psp = ctx.enter_context(tc.tile_pool(name="psp", bufs=2, space="PSUM"))
    uacc = ctx.enter_context(tc.tile_pool(name="uacc", bufs=1, space="PSUM"))
    utmp = ctx.enter_context(tc.tile_pool(name="utmp", bufs=2, space="PSUM"))
    psd = utmp

    # ---- constants ----
    bd_sb = const.tile([128, 128], BF16, tag="bd")
    nc.gpsimd.dma_start(bd_sb, bdG)
    id_sb = const.tile([128, 128], BF16, tag="id")
    nc.gpsimd.dma_start(id_sb, ident)
    zero_sb = const.tile([128, 128], BF16, tag="zero")
    nc.vector.memset(zero_sb, 0.0)

    def warmup(n):
        wt = utmp.tile([128, 64], F32, tag="ut", name="warm")
        for i in range(n):
            nc.tensor.matmul(wt, lhsT=zero_sb, rhs=zero_sb[:, 0:64],
                             start=True, stop=True)
    wo_sb = []
    for pc in range(4):
        wo_sb.append(const.tile([128, E], BF16, tag=f"wo{pc}", name=f"wo{pc}"))
    bo_sb = const.tile([128, E], F32, tag="bo")
    xq_sb = []
    for p in range(4):
        t = const.tile([128, QLEN], BF16, tag=f"xq{p}", name=f"xq{p}")
        xq_sb.append(t)

    mask_res = [const.tile([128, QLEN], BF16, tag=f"mk{c}", name=f"mk{c}")
                for c in range(KC)]
    valp_t = [const.tile([128, H * 65], BF16, tag=f"vp{c}", name=f"vp{c}")
              for c in range(KC)]

    def load_xq(p):
        nc.sync.dma_start(xq_sb[p], xqT[(2 * p) * DH:(2 * p + 2) * DH, :])

    def load_late_consts():
        for pc in range(4):
            nc.sync.dma_start(wo_sb[pc], woT[pc * 128:(pc + 1) * 128, :])
        nc.sync.dma_start(bo_sb, bo_b)

    def load_masks_valp():
        for c in range(KC):
            nc.gpsimd.dma_start(valp_t[c], valp[c * 128:(c + 1) * 128, :])
        for c in range(KC):
            nc.sync.dma_start(mask_res[c], maskT[c * 128:(c + 1) * 128, :])

    xk_sb = {}

    def load_xk(p):
        t = xkst.tile([128, S], BF16, tag="xk", name=f"xk{p}")
        for c in range(4):
            nc.sync.dma_start(t[:, c * 1024:(c + 1) * 1024],
                              xkT[(2 * p) * DH:(2 * p + 2) * DH,
                                  c * 1024:(c + 1) * 1024])
        xk_sb[p] = t

    qg_sb = {}

    def qg_chunks(p):
        """q-side projection with folded G: qgT = blockdiag(G,G)^T-applied."""
        qg_sb[p] = qgp.tile([128, QLEN], BF16, tag="qg", name=f"qg{p}")

        def one(qc):
            def go():
                t = utmp.tile([128, 512], F32, tag="ut", name=f"qgp{p}_{qc}")
                nc.tensor.matmul(t, lhsT=bd_sb,
                                 rhs=xq_sb[p][:, qc * 512:(qc + 1) * 512],
                                 start=True, stop=True)
                nc.vector.tensor_copy(
                    qg_sb[p][:, qc * 512:(qc + 1) * 512], t)
            return go
        return [one(0), one(1)]

    concatT = []
    for pair in range(4):
        concatT.append(const.tile([128, QLEN], BF16, tag=f"ct{pair}",
                                  name=f"ct{pair}"))

    def attn(pair, qw, trickle=(), last=False):
        trickle = list(trickle)
        dve_set = (2, 8, 14, 20) if last else (2, 8, 14, 20, 24)
        pool_set = (1, 5, 9, 15) if last else (1, 5, 9, 15, 19, 23)
        xk = xk_sb[pair]
        qg = qg_sb[pair]
        U = uacc.tile([128, 1024], F32, tag="u", name=f"U{pair}_{qw}")
        # PSUM accumulation groups are bank-granular (2KB zero regions), and
        # each U bank holds 4 sub-bank accumulators. Open the group with one
        # full-bank zero matmul (start=True); every AV matmul accumulates
        # with start=False (first touch reads pending-zero bytes as 0); a
        # full-bank zero matmul with stop=True closes the group, WAW-ordered
        # after all AV writes.
        for bk in range(2):
            nc.tensor.matmul(U[:, bk * 512:(bk + 1) * 512], lhsT=zero_sb,
                             rhs=valp_t[0][:, 0:512], start=True, stop=False)
        pts = {}

        def av(kc):
            pt = pts.pop(kc)
            for h2 in range(2):
                h = 2 * pair + h2
                for t in range(4):
                    slot = (h2 * 4 + t) * 128
                    nc.tensor.matmul(
                        U[:, slot:slot + 65],
                        lhsT=pt[:, h2 * 512 + t * 128:h2 * 512 + (t + 1) * 128],
                        rhs=valp_t[kc][:, h * 65:(h + 1) * 65],
                        start=False, stop=False)

        deferred = []
        next_av = [0]
        deferred_flush = True
        for kc in range(KC):
            dve_route = kc in dve_set
            pt = ptp.tile([128, 1024], BF16, tag="pt", name=f"pt{pair}_{qw}_{kc}")
            ms = mask_res[kc][:, qw * 512:(qw + 1) * 512]
            mb = bass.AP(tensor=ms.tensor, offset=ms.offset,
                         ap=[ms.ap[0], [0, 2], [1, 512]])
            pv = pt.rearrange("p (h q) -> p h q", h=2)
            if dve_route:
                # DVE route: scores land in the small psd pool so psp's only
                # consumer stays ACT; exp via poly + 7 squarings, mask fused
                pe = pep.tile([128, 1024], F32, tag="pe",
                              name=f"pe{pair}_{qw}_{kc}")
                for h2 in range(2):
                    psh = psd.tile([128, 512], F32, tag="ut",
                                   name=f"psd{pair}_{qw}_{kc}_{h2}")
                    nc.tensor.matmul(
                        psh,
                        lhsT=xk[h2 * DH:(h2 + 1) * DH, kc * 128:(kc + 1) * 128],
                        rhs=qg[h2 * DH:(h2 + 1) * DH, qw * 512:(qw + 1) * 512],
                        start=True, stop=True)
                    nc.vector._custom_dve(
                        op_poly, out=pe[:, h2 * 512:(h2 + 1) * 512], in0=psh,
                        s0=_EB1, s1=_EB2, imm2=_EB3)
                nc.vector._custom_dve(op_pow, out=pv,
                                      in0=pe.rearrange("p (h q) -> p h q", h=2),
                                      in1=mb)
            else:
                ps = psp.tile([128, 1024], F32, tag="ps",
                              name=f"ps{pair}_{qw}_{kc}")
                for h2 in range(2):
                    nc.tensor.matmul(
                        ps[:, h2 * 512:(h2 + 1) * 512],
                        lhsT=xk[h2 * DH:(h2 + 1) * DH, kc * 128:(kc + 1) * 128],
                        rhs=qg[h2 * DH:(h2 + 1) * DH, qw * 512:(qw + 1) * 512],
                        start=True, stop=True)
                nc.scalar.activation(pt, ps, AF.Exp, bias=0.0, scale=0.125)
                if kc in pool_set:
                    # Pool route for the mask; its AV is deferred to the
                    # phase tail so the PE stream never waits on Pool's FIFO
                    for h2 in range(2):
                        nc.gpsimd.tensor_mul(
                            pt[:, h2 * 512:(h2 + 1) * 512],
                            pt[:, h2 * 512:(h2 + 1) * 512], ms)
                    deferred.append(kc)
                else:
                    nc.vector.tensor_mul(pv, pv, mb)
            pts[kc] = pt
            budget = 2 if kc >= 26 else 1
            while budget > 0 and next_av[0] <= kc - 6:
                j = next_av[0]
                next_av[0] += 1
                if j not in deferred:
                    av(j)
                    budget -= 1
            if kc >= 27 and deferred_flush and deferred:
                av(deferred.pop(0))
            if trickle and kc >= 1:
                trickle.pop(0)()
        for k in [k for k in sorted(pts) if k not in deferred]:
            av(k)
        for k in list(deferred):
            av(k)
        for bk in range(2):
            nc.tensor.matmul(U[:, bk * 512:(bk + 1) * 512], lhsT=zero_sb,
                             rhs=valp_t[0][:, 0:512], start=False, stop=True)
        for work in trickle:
            work()

        # U-reading normalization emitted NOW (before the next phase reuses
        # the single U buffer); U-independent transpose+copy returned as
        # closures to trickle into the next phase.
        uns = {}
        for h2 in range(2):
            for t in range(4):
                slot = (h2 * 4 + t) * 128
                rc = rcp.tile([128, 1], F32, tag="rc",
                              name=f"rc{pair}_{qw}_{h2}_{t}")
                nc.vector.reciprocal(rc, U[:, slot + 64:slot + 65])
                un = unp.tile([128, 64], BF16, tag="un",
                              name=f"un{pair}_{qw}_{h2}_{t}")
                nc.vector.tensor_scalar_mul(un, U[:, slot:slot + 64], rc)
                uns[(h2, t)] = un

        def tr_one(h2, t):
            def go():
                ut = utmp.tile([64, 128], BF16, tag="ut",
                               name=f"utr{pair}_{qw}_{h2}_{t}")
                nc.tensor.transpose(ut, uns[(h2, t)], id_sb)
                nc.vector.tensor_copy(
                    concatT[pair][h2 * 64:(h2 + 1) * 64,
                                  qw * 512 + t * 128:qw * 512 + (t + 1) * 128],
                    ut)
            return go

        return [tr_one(h2, t) for h2 in range(2) for t in range(4)]

    def outproj(qts):
        def one(qt):
            def go():
                op = utmp.tile([128, 512], F32, tag="ut", name=f"op{qt}")
                for pc in range(4):
                    nc.tensor.matmul(op,
                                     lhsT=concatT[pc][:, qt * 128:(qt + 1) * 128],
                                     rhs=wo_sb[pc],
                                     start=(pc == 0), stop=(pc == 3))
                osb = ospool.tile([128, E], F32, tag="osb", name=f"osb{qt}")
                nc.vector.scalar_tensor_tensor(osb, op, 1.0, bo_sb,
                                               ALU.mult, ALU.add)
                nc.sync.dma_start(out[qt * 128:(qt + 1) * 128, :], osb)
            return go
        return [one(qt) for qt in qts]

    # ---- emission schedule ----
    warmup(56)
    load_xq(0)
    load_xk(0)
    for c in range(KC):
        nc.sync.dma_start(mask_res[c], maskT[c * 128:(c + 1) * 128, :])
        nc.gpsimd.dma_start(valp_t[c], valp[c * 128:(c + 1) * 128, :])
        if c == 16:
            load_xq(1)
    load_xk(1)
    load_xq(2)
    load_xq(3)
    load_xk(2)
    load_xk(3)
    load_late_consts()
    for work in qg_chunks(0):
        work()
    t00 = attn(0, 0)
    t01 = attn(0, 1, trickle=t00 + qg_chunks(1))
    t10 = attn(1, 0, trickle=t01)
    t11 = attn(1, 1, trickle=t10 + qg_chunks(2))
    t20 = attn(2, 0, trickle=t11)
    t21 = attn(2, 1, trickle=t20 + qg_chunks(3))
    t30 = attn(3, 0, trickle=t21)
    t31 = attn(3, 1, trickle=t30 + outproj(range(4)), last=True)
    qts = outproj(range(4, 8))
    # tail: as soon as both heads' norms for q-tile t land, fire its outproj
    for t in range(4):
        t31[t]()
        t31[4 + t]()
        qts[t]()

    ctx.close()


def _prep_inputs(key, query, value, mask, Wq, Wk, Wv, Wo, bo):
    bf16 = ml_dtypes.bfloat16
    key = np.asarray(key, np.float32)
    query = np.asarray(query, np.float32)
    value = np.asarray(value, np.float32)
    mask = np.asarray(mask)
    Wq = np.asarray(Wq, np.float64)
    Wk = np.asarray(Wk, np.float64)
    Wv = np.asarray(Wv, np.float64)
    Wo = np.asarray(Wo, np.float64)

    G = Wq.T @ Wk                       # scores = (Xq G) Xk^T
    BD = np.zeros((128, 128), np.float64)
    BD[:64, :64] = G
    BD[64:, 64:] = G
    WoT = Wo.T                          # [in 512, out 512]
    woT_eff = np.concatenate(
        [Wv.T @ WoT[h * DH:(h + 1) * DH, :] for h in range(H)], axis=0)

    common = {
        "bdG": np.ascontiguousarray(BD.astype(np.float32)).astype(bf16),
        "ident": np.eye(128, dtype=np.float32).astype(bf16),
        "woT": np.ascontiguousarray(woT_eff.astype(np.float32)).astype(bf16),
        "bo_b": np.ascontiguousarray(
            np.broadcast_to(np.asarray(bo, np.float32), (128, E))),
    }
    maskT = np.ascontiguousarray(
        (mask[0, 0] != 0).astype(np.float32).T.astype(bf16))  # [k, q]
    per_b = {}
    for b in range(B):
        vp = np.ones((S, H, 65), np.float32)
        vp[:, :, :64] = value[b].reshape(S, H, DH)
        per_b[b] = {
            "xkT": np.ascontiguousarray(key[b].T).astype(bf16),
            "valp": np.ascontiguousarray(vp.reshape(S, H * 65).astype(bf16)),
            "qT": query[b].T,
        }
    in_maps = []
    for c in range(N_CORES):
        b, qs = c // 4, (c % 4) * QLEN
        in_maps.append({
            "xqT": np.ascontiguousarray(
                per_b[b]["qT"][:, qs:qs + QLEN]).astype(bf16),
            "xkT": per_b[b]["xkT"],
            "valp": per_b[b]["valp"],
            "maskT": np.ascontiguousarray(maskT[:, qs:qs + QLEN]),
            **common,
        })
    return in_maps


def get_module():
    if "nc" not in _CACHE:
        _CACHE["nc"] = _build_module()
    return _CACHE["nc"]


def kernel(key, query, value, mask, Wq, Wk, Wv, Wo, bo, **_):
    nc = get_module()
    in_maps = _prep_inputs(key, query, value, mask, Wq, Wk, Wv, Wo, bo)
    res = bass_utils.run_bass_kernel_spmd(
        nc, in_maps, core_ids=list(range(N_CORES)))
    full = np.empty((B, S, E), np.float32)
    for c in range(N_CORES):
        b, qs = c // 4, (c % 4) * QLEN
        full[b, qs:qs + QLEN, :] = res.results[c]["out"]
    return full
